# revision 45
# baseline (speedup 1.0000x reference)
"""GQA attention (B=2, S=2048, D=2048, 32 q-heads / 8 kv-heads, hd=64),
tensor-parallel over the 8 kv-head groups on 8 NeuronCores.

Per-core math (core c owns kv head c and q heads 4c..4c+3):
  qT = (wq_c @ x.T), kT/vT likewise; RoPE via elementwise muls with a
  partition pair-swap done by DVE stream_shuffle and a sign-folded sin
  table; scoresT[sk,sq] = k_rot.T-layout matmul; causal masking folded
  into the scores PSUM accumulation via a pair of constant triangular
  matrices (ltri.T@utri adds -32768*max(0,sk-sq) on the 128-wide
  diagonal band) so exp output needs no post-masking; ET = exp(scoresT/8);
  out_pvT and the softmax denominator come from one matmul against
  [V | ones]; partial = attnT.T @ woT_c accumulated in fp32 and summed
  on host.

Scheduling: the PE runs its queue in order and downclocks (p-state)
whenever it idles, so the kernel aims for long contiguous PE streaks:
 - scores are software-pipelined 2 steps ahead of PV (4 PSUM banks),
 - the Scalar engine's exp throughput deficit (~0.3us per score step)
   is absorbed by pulling coarse chunks of projection/output-projection
   matmuls from a filler queue every 4th step,
 - projections run as two passes (Q then KV) over the cached x tiles so
   they need only 2 PSUM banks, shared with the output projection.
"""

from collections import deque
from contextlib import ExitStack

import ml_dtypes
import numpy as np

import concourse.bass as bass
import concourse.tile as tile
from concourse import bacc, mybir
from concourse import bass_utils
from concourse.bass_interp import get_hw_module

BF16 = mybir.dt.bfloat16
F32 = mybir.dt.float32
F32R = mybir.dt.float32r

N_CORES = 8
B, S, DIM = 2, 2048, 2048
NH, NKV, HD = 32, 8, 64          # global heads
NHC = NH // N_CORES              # q heads per core = 4
QD = NHC * HD                    # per-core q out dim = 256
ST = B * S                       # total tokens = 4096
KT = DIM // 128                  # contraction k-tiles = 16
SQT = 512                        # sq tile (matmul free dim)
SKT = 128                        # sk tile (partition dim)
NSQ = S // SQT                   # sq tiles per batch = 4
NSK = S // SKT                   # sk tiles per batch = 16

SWAP32 = [i ^ 1 for i in range(32)]  # pair-swap within 32-partition groups

_CACHE: dict = {}


def _build():
    if "nc" in _CACHE:
        return _CACHE["nc"]
    nc = bacc.Bacc(
        "TRN2",
        target_bir_lowering=False,
        debug=False,
        enable_asserts=False,
        num_devices=N_CORES,
    )
    xT = nc.dram_tensor("xt", [DIM, ST], BF16, kind="ExternalInput").ap()
    wqT = nc.dram_tensor("wqt", [DIM, QD], BF16, kind="ExternalInput").ap()
    wkvT = nc.dram_tensor("wkvt", [DIM, 2 * HD], BF16, kind="ExternalInput").ap()
    woT = nc.dram_tensor("wot", [QD, DIM], BF16, kind="ExternalInput").ap()
    cosE = nc.dram_tensor("cose", [128, ST], BF16, kind="ExternalInput").ap()
    sinE = nc.dram_tensor("sine", [128, ST], BF16, kind="ExternalInput").ap()
    ident = nc.dram_tensor("ident", [64, 64], BF16, kind="ExternalInput").ap()
    ltri = nc.dram_tensor("ltri", [128, 128], BF16, kind="ExternalInput").ap()
    utri = nc.dram_tensor("utri", [128, 128], BF16, kind="ExternalInput").ap()
    ones64 = nc.dram_tensor("ones64", [1, 64], F32, kind="ExternalInput").ap()
    out = nc.dram_tensor("out", [ST, DIM], BF16, kind="ExternalOutput").ap()

    with tile.TileContext(nc) as tc, ExitStack() as ctx:
        pers = ctx.enter_context(tc.tile_pool(name="pers", bufs=1))

        # -- persistent SBUF tensors ------------------------------------
        wq_ch = [pers.tile([128, 4 * QD], BF16, tag=f"wq{g}", name=f"wq{g}")
                 for g in range(4)]
        wkv_ch = [pers.tile([128, 4 * 2 * HD], BF16, tag=f"wkv{g}",
                            name=f"wkv{g}") for g in range(4)]
        wo_sb = [pers.tile([128, DIM], BF16, tag=f"wo{j}", name=f"wo{j}") for j in range(2)]
        cos_sb = pers.tile([128, ST], BF16, tag="cos")
        sin_sb = pers.tile([128, ST], BF16, tag="sin")
        id_sb = pers.tile([64, 64], BF16, tag="ident")
        ltri_sb = pers.tile([128, 128], BF16, tag="ltri")
        utri_sb = pers.tile([128, 128], BF16, tag="utri")
        ones64_sb = pers.tile([1, 64], F32, tag="ones64")
        qrot = [pers.tile([128, ST], BF16, tag=f"qrot{t}", name=f"qrot{t}") for t in range(2)]
        # zero-padded stationary K: krot2[hp] holds k_rot in partition half
        # hp and zeros in the other, so scores run as full K=128 matmuls
        # (K=64 quadrant matmuls pay a ~106ns fixed cost on TRN2)
        krot2 = [pers.tile([128, ST], BF16, tag=f"krot{t}", name=f"krot{t}") for t in range(2)]
        vaug = pers.tile([128, B * NSK * 65], BF16, tag="vaug")
        attnT = [pers.tile([128, ST], BF16, tag=f"attnT{t}", name=f"attnT{t}") for t in range(2)]

        # weights first: the first projection matmuls gate kernel start
        wqT_v = wqT.rearrange("(t p) d -> p t d", p=128)
        wkvT_v = wkvT.rearrange("(t p) d -> p t d", p=128)
        for g in range(4):
            gs = slice(g * 4, (g + 1) * 4)
            nc.sync.dma_start(
                wq_ch[g].rearrange("p (t d) -> p t d", t=4), wqT_v[:, gs, :]
            )
            nc.sync.dma_start(
                wkv_ch[g].rearrange("p (t d) -> p t d", t=4), wkvT_v[:, gs, :]
            )
        def late_dmas():
            # ones column of V_aug (col 64 of each 65-wide block); these
            # memsets go after the first x-tile DMA triggers on the gpsimd
            # queue so they don't delay the kernel start
            nc.gpsimd.memset(vaug[:, 64::65], 1.0)
            nc.gpsimd.memset(krot2[0][64:128, :], 0.0)
            nc.gpsimd.memset(krot2[1][0:64, :], 0.0)
            # issued after the first x tiles so they don't delay the start
            for j in range(2):
                nc.sync.dma_start(wo_sb[j][:], woT[j * 128:(j + 1) * 128, :])
            for g in range(4):
                gs = bass.ts(g, ST // 4)
                nc.sync.dma_start(cos_sb[:, gs], cosE[:, gs])
                nc.sync.dma_start(sin_sb[:, gs], sinE[:, gs])
            nc.sync.dma_start(id_sb[:], ident[:])
            nc.sync.dma_start(ltri_sb[:], ltri[:])
            nc.sync.dma_start(utri_sb[:], utri[:])
            nc.sync.dma_start(ones64_sb[:], ones64[:])

        # -- pools -------------------------------------------------------
        # PSUM (8 banks): proj 2 (A projections + C output proj),
        #                 sc 4 (scores pipeline depth 2), po 2 (PV accum)
        with tc.tile_pool(name="xt", bufs=32) as xp, \
             tc.tile_pool(name="stage", bufs=2) as sp, \
             tc.tile_pool(name="et", bufs=6) as ep, \
             tc.tile_pool(name="misc", bufs=2) as mp, \
             tc.tile_pool(name="wout", bufs=8) as woutp, \
             tc.tile_pool(name="ps8", bufs=1, space="PSUM") as pool8:

            xt_cache = {}
            filler_q = deque()  # (pe_weight, thunk) of deferred work

            def pump(units):
                while units > 0 and filler_q:
                    w, f = filler_q.popleft()
                    f()
                    units -= w

            def pump_all():
                while filler_q:
                    _, f = filler_q.popleft()
                    f()

            def queue_a(st):
                """Queue projection+rope work for seq tile st as fillers.

                All PSUM allocation happens lazily inside thunks so
                tag-slot round-robin order matches PE emission order."""
                ss = bass.ts(st, SQT)
                h = {}
                xt_cache.clear()
                for kt in range(KT):
                    t = xp.tile([128, SQT], BF16, name="xt_t")
                    nc.sync.dma_start(
                        t[:], xT[kt * 128:(kt + 1) * 128,
                                 st * SQT:(st + 1) * SQT]
                    )
                    xt_cache[kt] = t

                def mk_q(kt):
                    xt_t = xt_cache[kt]

                    def f():
                        if kt == 0:
                            h["psq"] = [pool8.tile([128, SQT], F32, tag="proj",
                                                   name="psq", bufs=2)
                                        for _ in range(2)]
                        psq = h["psq"]
                        for dt in range(2):
                            nc.tensor.matmul(
                                psq[dt][:],
                                wq_ch[kt // 4][:, (kt % 4) * QD + dt * 128:
                                               (kt % 4) * QD + (dt + 1) * 128],
                                xt_t[:],
                                start=(kt == 0),
                                stop=(kt == KT - 1),
                            )
                    return f

                for kt in range(KT):
                    filler_q.append((2, mk_q(kt)))

                def mk_rope(dt):
                    def f():
                        psq = h["psq"]
                        qsb = sp.tile([128, SQT], BF16, tag="qsb", name="qsb")
                        nc.vector.tensor_copy(qsb[:], psq[dt][:])
                        qsw = sp.tile([128, SQT], BF16, tag="qsw", name="qsw")
                        nc.vector.stream_shuffle(qsw[:], qsb[:], SWAP32)
                        t1 = sp.tile([128, SQT], BF16, tag="t1", name="t1")
                        nc.vector.tensor_mul(t1[:], qsb[:], cos_sb[:, ss])
                        t2 = sp.tile([128, SQT], BF16, tag="t2", name="t2")
                        nc.vector.tensor_mul(t2[:], qsw[:], sin_sb[:, ss])
                        nc.vector.tensor_add(qrot[dt][:, ss], t1[:], t2[:])
                    return f

                for dt in range(2):
                    filler_q.append((0, mk_rope(dt)))

                def mk_kv(kt):
                    xt_t = xt_cache[kt]

                    def f():
                        if kt == 0:
                            h["pskv"] = pool8.tile([128, SQT], F32, tag="proj",
                                                   name="pskv", bufs=2)
                        nc.tensor.matmul(
                            h["pskv"][:],
                            wkv_ch[kt // 4][:, (kt % 4) * 128:(kt % 4 + 1) * 128],
                            xt_t[:],
                            start=(kt == 0),
                            stop=(kt == KT - 1),
                        )
                    return f

                for kt in range(KT):
                    filler_q.append((1, mk_kv(kt)))

                def rope_k():
                    pskv = h["pskv"]
                    ksb = sp.tile([64, SQT], BF16, tag="ksb", name="ksb")
                    nc.vector.tensor_copy(ksb[:], pskv[0:64, :])
                    ksw = sp.tile([64, SQT], BF16, tag="ksw", name="ksw")
                    nc.vector.stream_shuffle(ksw[:], ksb[:], SWAP32)
                    t1k = sp.tile([64, SQT], BF16, tag="t1k", name="t1k")
                    nc.vector.tensor_mul(t1k[:], ksb[:], cos_sb[0:64, ss])
                    t2k = sp.tile([64, SQT], BF16, tag="t2k", name="t2k")
                    nc.vector.tensor_mul(t2k[:], ksw[:], sin_sb[0:64, ss])
                    nc.vector.tensor_add(krot2[0][0:64, ss], t1k[:], t2k[:])
                    nc.vector.tensor_add(krot2[1][64:128, ss], t1k[:], t2k[:])

                filler_q.append((0, rope_k))

                def vtrans():
                    pskv = h["pskv"]
                    vsb = sp.tile([64, SQT], BF16, tag="vsb", name="vsb")
                    nc.vector.tensor_copy(vsb[:], pskv[64:128, :])
                    for c in range(SQT // 128):
                        j = st * 4 + c  # global sk tile index
                        pt = pool8.tile([128, 64], BF16, tag="proj", name="pt",
                                        bufs=2)
                        nc.tensor.transpose(
                            pt[:], vsb[:, c * 128:(c + 1) * 128], id_sb[:]
                        )
                        nc.vector.tensor_copy(vaug[:, j * 65: j * 65 + 64], pt[:])

                filler_q.append((1, vtrans))

            def queue_c(b, sqt):
                """Queue output-projection work for block (b,sqt) as fillers."""
                for sti in range(SQT // 128):
                    st = (b * S + sqt * SQT) // 128 + sti
                    for ot in range(DIM // SQT):
                        def mk(st=st, ot=ot, sti=sti):
                            def f():
                                pw = pool8.tile([128, SQT], F32, tag="proj",
                                                name="pw", bufs=2)
                                for jt in range(2):
                                    nc.tensor.matmul(
                                        pw[:],
                                        attnT[jt][:, st * 128:(st + 1) * 128],
                                        wo_sb[jt][:, ot * SQT:(ot + 1) * SQT],
                                        start=(jt == 0),
                                        stop=(jt == 1),
                                    )
                                osb = woutp.tile([128, SQT], BF16, tag="osb",
                                                 name="osb")
                                if (sti + ot) % 2 == 0:
                                    nc.scalar.copy(osb[:], pw[:])
                                else:
                                    nc.vector.tensor_copy(osb[:], pw[:])
                                nc.sync.dma_start(
                                    out[st * 128:(st + 1) * 128,
                                        ot * SQT:(ot + 1) * SQT],
                                    osb[:],
                                )
                            return f
                        filler_q.append((2, mk()))

            def emit_b(b, sqt):
                n_sk = 4 * (sqt + 1)
                sq0 = b * S + sqt * SQT
                for dt in range(2):  # head pair (hp=0,1 packed in PE halves)
                    po = [pool8.tile([128, SQT], F32, tag="po",
                                     name=f"po{hp}", bufs=2) for hp in range(2)]
                    pend = deque()  # pipelined PV args, depth 2

                    def emit_pv(ets, j, off):
                        jj = b * NSK + j
                        for hp in range(2):
                            nc.tensor.matmul(
                                po[hp][0:65, off:SQT],
                                vaug[:, jj * 65:(jj + 1) * 65],
                                ets[hp][:, off:SQT],
                                start=(j == 0),
                                stop=(j == n_sk - 1),
                            )

                    for j in range(n_sk):
                        sk0 = b * S + j * SKT
                        d = j - 4 * sqt
                        off = max(0, 128 * d)  # causally dead columns
                        pss = []
                        for hp in range(2):
                            ps = pool8.tile([128, SQT], F32, tag="sc",
                                            name="ps", bufs=4)
                            nc.tensor.matmul(
                                ps[:, off:SQT],
                                krot2[hp][:, sk0:sk0 + SKT],
                                qrot[dt][:, sq0 + off:sq0 + SQT],
                                start=True,
                                stop=(d < 0),
                            )
                            if d >= 0:
                                # fold causal mask into PSUM: adds
                                # -32768*max(0, sk-sq) on the 128-wide band
                                nc.tensor.matmul(
                                    ps[:, off:off + 128],
                                    ltri_sb[:],
                                    utri_sb[:],
                                    start=False,
                                    stop=True,
                                    skip_group_check=True,
                                )
                            pss.append(ps)
                        ets = []
                        for hp in range(2):
                            et = ep.tile([128, SQT], BF16, tag=f"et{hp}",
                                         name=f"et{hp}")
                            nc.scalar.activation(
                                et[:, off:SQT], pss[hp][:, off:SQT],
                                mybir.ActivationFunctionType.Exp,
                                scale=0.125,
                            )
                            ets.append(et)
                        pend.append((ets, j, off))
                        if j % 4 == 3:
                            pump(6)
                        if len(pend) > 2:
                            emit_pv(*pend.popleft())
                    while pend:
                        pump(3)
                        emit_pv(*pend.popleft())
                    for hp in range(2):
                        den = mp.tile([1, SQT], F32, tag="den", name="den")
                        nc.vector.tensor_copy(den[:], po[hp][64:65, :])
                        recip = mp.tile([1, SQT], F32, tag="recip", name="recip")
                        nc.vector.reciprocal_approx_fast(recip[:], den[:])
                        bc = mp.tile([64, SQT], F32, tag="bc", name="bc")
                        nc.gpsimd.partition_broadcast(bc[:], recip[:])
                        nc.vector.tensor_mul(
                            attnT[dt][hp * 64:(hp + 1) * 64, sq0:sq0 + SQT],
                            po[hp][0:64, :],
                            bc[:],
                        )

            # schedule: A(st) queued TWO blocks ahead of the B block whose
            # scores read its qrot, so the vector rope chain of a drained A
            # has a whole block of slack before the PE depends on it.
            queue_a(0)
            late_dmas()
            pump_all()
            queue_a(1)
            pump_all()
            queue_a(2); emit_b(0, 0); pump_all(); queue_c(0, 0)
            queue_a(3); emit_b(0, 1); pump_all(); queue_c(0, 1)
            queue_a(4); emit_b(0, 2); pump_all(); queue_c(0, 2)
            queue_a(5); emit_b(0, 3); pump_all(); queue_c(0, 3)
            queue_a(6); emit_b(1, 0); pump_all(); queue_c(1, 0)
            queue_a(7); emit_b(1, 1); pump_all(); queue_c(1, 1)
            emit_b(1, 2); queue_c(1, 2)
            emit_b(1, 3)
            queue_c(1, 3)
            pump_all()

    nc.compile()
    nc.m = get_hw_module(nc.m)
    _CACHE["nc"] = nc
    return nc


def _prep_inputs(x, freqs_cos, freqs_sin, wq, wk, wv, wo):
    bf = ml_dtypes.bfloat16
    xT = np.ascontiguousarray(x.reshape(ST, DIM).T).astype(bf)
    # expanded rope tables in [feature, seq] layout, tiled over 2 head rows;
    # sin carries the pair-swap signs (-sin on even rows, +sin on odd)
    cos64 = np.repeat(freqs_cos.T, 2, axis=0)        # [64, S]
    sin64 = np.repeat(freqs_sin.T, 2, axis=0)
    sgn = np.tile(np.array([-1.0, 1.0]), 32)[:, None]
    sin64 = sin64 * sgn
    cosE = np.tile(np.tile(cos64, (2, 1)), (1, B)).astype(bf)  # [128, ST]
    sinE = np.tile(np.tile(sin64, (2, 1)), (1, B)).astype(bf)
    ident = np.eye(64, dtype=np.float32).astype(bf)
    # causal-mask factor pair: (ltri.T @ utri)[m, n] = -32768*max(0, m-n)
    rr = np.arange(128)
    ltri = (rr[:, None] <= rr[None, :]).astype(np.float32).astype(bf)  # r<=m
    utri = ((rr[:, None] > rr[None, :]) * -32768.0).astype(np.float32).astype(bf)
    ones64 = np.ones((1, 64), dtype=np.float32)

    in_maps = []
    for c in range(N_CORES):
        wq_c = wq[c * QD:(c + 1) * QD, :]
        wk_c = wk[c * HD:(c + 1) * HD, :]
        wv_c = wv[c * HD:(c + 1) * HD, :]
        wkv_c = np.concatenate([wk_c, wv_c], axis=0)   # [128, DIM]
        wo_c = wo[:, c * QD:(c + 1) * QD]              # [DIM, 256]
        in_maps.append({
            "xt": xT,
            "wqt": np.ascontiguousarray(wq_c.T).astype(bf),
            "wkvt": np.ascontiguousarray(wkv_c.T).astype(bf),
            "wot": np.ascontiguousarray(wo_c.T).astype(bf),
            "cose": cosE,
            "sine": sinE,
            "ident": ident,
            "ltri": ltri,
            "utri": utri,
            "ones64": ones64,
        })
    return in_maps


def kernel(x, freqs_cos, freqs_sin, wq, wk, wv, wo, _trace=False, _trace_kwargs=None):
    x = np.asarray(x, dtype=np.float32)
    freqs_cos = np.asarray(freqs_cos, dtype=np.float32)
    freqs_sin = np.asarray(freqs_sin, dtype=np.float32)
    wq = np.asarray(wq, dtype=np.float32)
    wk = np.asarray(wk, dtype=np.float32)
    wv = np.asarray(wv, dtype=np.float32)
    wo = np.asarray(wo, dtype=np.float32)

    nc = _build()
    in_maps = _prep_inputs(x, freqs_cos, freqs_sin, wq, wk, wv, wo)
    kwargs = dict(_trace_kwargs or {})
    res = bass_utils.run_bass_kernel_spmd(
        nc, in_maps, core_ids=list(range(N_CORES)), trace=_trace, **kwargs
    )
    _CACHE["last_result"] = res
    acc = res.results[0]["out"].astype(np.float32)
    for c in range(1, N_CORES):
        acc += res.results[c]["out"].astype(np.float32)
    return acc.reshape(B, S, DIM)


# revision 51
# speedup vs baseline: 1.1499x; 1.1499x over previous
"""GQA attention (B=2, S=2048, D=2048, 32 q-heads / 8 kv-heads, hd=64),
tensor-parallel over the 8 kv-head groups on 8 NeuronCores.

Per-core math (core c owns kv head c and q heads 4c..4c+3):
  qT = (wq_c @ x.T), kT/vT likewise; RoPE via elementwise muls with a
  partition pair-swap done by DVE stream_shuffle and a sign-folded sin
  table; scoresT[sk,sq] = k_rot.T-layout matmul; causal masking folded
  into the scores PSUM accumulation via a pair of constant triangular
  matrices (ltri.T@utri adds -32768*max(0,sk-sq) on the 128-wide
  diagonal band) so exp output needs no post-masking; ET = exp(scoresT/8);
  out_pvT and the softmax denominator come from one matmul against
  [V | ones]; partial = attnT.T @ woT_c accumulated in fp32 and summed
  on host.

Scheduling: the PE runs its queue in order and downclocks (p-state)
whenever it idles, so the kernel aims for long contiguous PE streaks:
 - scores are software-pipelined 2 steps ahead of PV (4 PSUM banks),
 - the Scalar engine's exp throughput deficit (~0.3us per score step)
   is absorbed by pulling coarse chunks of projection/output-projection
   matmuls from a filler queue every 4th step,
 - projections run as two passes (Q then KV) over the cached x tiles so
   they need only 2 PSUM banks, shared with the output projection.
"""

from collections import deque
from contextlib import ExitStack

import ml_dtypes
import numpy as np

import concourse.bass as bass
import concourse.tile as tile
from concourse import bacc, mybir
from concourse import bass_utils
from concourse.bass_interp import get_hw_module

BF16 = mybir.dt.bfloat16
F32 = mybir.dt.float32
F32R = mybir.dt.float32r

N_CORES = 8
B, S, DIM = 2, 2048, 2048
NH, NKV, HD = 32, 8, 64          # global heads
NHC = NH // N_CORES              # q heads per core = 4
QD = NHC * HD                    # per-core q out dim = 256
ST = B * S                       # total tokens = 4096
KT = DIM // 128                  # contraction k-tiles = 16
SQT = 512                        # sq tile (matmul free dim)
SKT = 128                        # sk tile (partition dim)
NSQ = S // SQT                   # sq tiles per batch = 4
NSK = S // SKT                   # sk tiles per batch = 16

SWAP32 = [i ^ 1 for i in range(32)]  # pair-swap within 32-partition groups

_CACHE: dict = {}


def _build():
    if "nc" in _CACHE:
        return _CACHE["nc"]
    nc = bacc.Bacc(
        "TRN2",
        target_bir_lowering=False,
        debug=False,
        enable_asserts=False,
        num_devices=N_CORES,
    )
    xT = nc.dram_tensor("xt", [DIM, ST], BF16, kind="ExternalInput").ap()
    wqT = nc.dram_tensor("wqt", [DIM, QD], BF16, kind="ExternalInput").ap()
    wkvT = nc.dram_tensor("wkvt", [DIM, 2 * HD], BF16, kind="ExternalInput").ap()
    woT = nc.dram_tensor("wot", [QD, DIM], BF16, kind="ExternalInput").ap()
    cosE = nc.dram_tensor("cose", [128, ST], BF16, kind="ExternalInput").ap()
    sinE = nc.dram_tensor("sine", [128, ST], BF16, kind="ExternalInput").ap()
    ident = nc.dram_tensor("ident", [64, 64], BF16, kind="ExternalInput").ap()
    ltri = nc.dram_tensor("ltri", [128, 128], BF16, kind="ExternalInput").ap()
    utri = nc.dram_tensor("utri", [128, 128], BF16, kind="ExternalInput").ap()
    ones64 = nc.dram_tensor("ones64", [1, 64], F32, kind="ExternalInput").ap()
    out = nc.dram_tensor("out", [ST, DIM], BF16, kind="ExternalOutput").ap()

    with tile.TileContext(nc) as tc, ExitStack() as ctx:
        pers = ctx.enter_context(tc.tile_pool(name="pers", bufs=1))

        # -- persistent SBUF tensors ------------------------------------
        wq_ch = [pers.tile([128, 4 * QD], BF16, tag=f"wq{g}", name=f"wq{g}")
                 for g in range(4)]
        wkv_ch = [pers.tile([128, 4 * 2 * HD], BF16, tag=f"wkv{g}",
                            name=f"wkv{g}") for g in range(4)]
        wo_sb = [pers.tile([128, DIM], BF16, tag=f"wo{j}", name=f"wo{j}") for j in range(2)]
        cos_sb = pers.tile([128, ST], BF16, tag="cos")
        sin_sb = pers.tile([128, ST], BF16, tag="sin")
        id_sb = pers.tile([64, 64], BF16, tag="ident")
        ltri_sb = pers.tile([128, 128], BF16, tag="ltri")
        utri_sb = pers.tile([128, 128], BF16, tag="utri")
        ones64_sb = pers.tile([1, 64], F32, tag="ones64")
        qrot = [pers.tile([128, ST], BF16, tag=f"qrot{t}", name=f"qrot{t}") for t in range(2)]
        # zero-padded stationary K: krot2[hp] holds k_rot in partition half
        # hp and zeros in the other, so scores run as full K=128 matmuls
        # (K=64 quadrant matmuls pay a ~106ns fixed cost on TRN2)
        krot2 = [pers.tile([128, ST], BF16, tag=f"krot{t}", name=f"krot{t}") for t in range(2)]
        vaug = pers.tile([128, B * NSK * 65], BF16, tag="vaug")
        warm_src = pers.tile([1, 8], F32, tag="warm_src")
        warm_dst = pers.tile([2, 8], F32, tag="warm_dst")
        attnT = [pers.tile([128, ST], BF16, tag=f"attnT{t}", name=f"attnT{t}") for t in range(2)]

        # weights first: the first projection matmuls gate kernel start
        wqT_v = wqT.rearrange("(t p) d -> p t d", p=128)
        wkvT_v = wkvT.rearrange("(t p) d -> p t d", p=128)
        for g in range(4):
            gs = slice(g * 4, (g + 1) * 4)
            nc.sync.dma_start(
                wq_ch[g].rearrange("p (t d) -> p t d", t=4), wqT_v[:, gs, :]
            )
            nc.sync.dma_start(
                wkv_ch[g].rearrange("p (t d) -> p t d", t=4), wkvT_v[:, gs, :]
            )
        nc.gpsimd.memset(warm_src[:], 1.0)

        def late_dmas():
            # ones column of V_aug (col 64 of each 65-wide block); these
            # memsets go after the first x-tile DMA triggers on the gpsimd
            # queue so they don't delay the kernel start
            nc.gpsimd.memset(vaug[:, 64::65], 1.0)
            nc.gpsimd.memset(krot2[0][64:128, :], 0.0)
            nc.gpsimd.memset(krot2[1][0:64, :], 0.0)
            # issued after the first x tiles so they don't delay the start
            for j in range(2):
                nc.sync.dma_start(wo_sb[j][:], woT[j * 128:(j + 1) * 128, :])
            for g in range(4):
                gs = bass.ts(g, ST // 4)
                nc.sync.dma_start(cos_sb[:, gs], cosE[:, gs])
                nc.sync.dma_start(sin_sb[:, gs], sinE[:, gs])
            nc.sync.dma_start(id_sb[:], ident[:])
            nc.sync.dma_start(ltri_sb[:], ltri[:])
            nc.sync.dma_start(utri_sb[:], utri[:])
            nc.sync.dma_start(ones64_sb[:], ones64[:])

        # -- pools -------------------------------------------------------
        # PSUM (8 banks): proj 2 (A projections + C output proj),
        #                 sc 4 (scores pipeline depth 2), po 2 (PV accum)
        with tc.tile_pool(name="xt", bufs=32) as xp, \
             tc.tile_pool(name="stage", bufs=2) as sp, \
             tc.tile_pool(name="et", bufs=6) as ep, \
             tc.tile_pool(name="misc", bufs=2) as mp, \
             tc.tile_pool(name="wout", bufs=8) as woutp, \
             tc.tile_pool(name="ps8", bufs=1, space="PSUM") as pool8:

            xt_cache = {}
            filler_q = deque()  # (pe_weight, thunk) of deferred work

            def pump(units):
                while units > 0 and filler_q:
                    w, f = filler_q.popleft()
                    f()
                    units -= w

            def pump_all():
                while filler_q:
                    _, f = filler_q.popleft()
                    f()

            def queue_a(st):
                """Queue projection+rope work for seq tile st as fillers.

                All PSUM allocation happens lazily inside thunks so
                tag-slot round-robin order matches PE emission order."""
                ss = bass.ts(st, SQT)
                h = {}
                xt_cache.clear()
                for kt in range(KT):
                    t = xp.tile([128, SQT], BF16, name="xt_t")
                    nc.gpsimd.dma_start(
                        t[:], xT[kt * 128:(kt + 1) * 128,
                                 st * SQT:(st + 1) * SQT]
                    )
                    xt_cache[kt] = t
                # pre-warm the gpsimd partition_broadcast library: the queue
                # alternates DMA-trigger and broadcast code, and the ~5us
                # library reload otherwise lands on the po-reuse critical
                # path at the block's first normalize
                nc.gpsimd.partition_broadcast(warm_dst[:], warm_src[:])

                def mk_q(kt):
                    xt_t = xt_cache[kt]

                    def f():
                        if kt == 0:
                            h["psq"] = [pool8.tile([128, SQT], F32, tag="proj",
                                                   name="psq", bufs=2)
                                        for _ in range(2)]
                        psq = h["psq"]
                        for dt in range(2):
                            nc.tensor.matmul(
                                psq[dt][:],
                                wq_ch[kt // 4][:, (kt % 4) * QD + dt * 128:
                                               (kt % 4) * QD + (dt + 1) * 128],
                                xt_t[:],
                                start=(kt == 0),
                                stop=(kt == KT - 1),
                            )
                    return f

                for kt in range(KT):
                    filler_q.append((2, mk_q(kt)))

                def mk_rope(dt):
                    def f():
                        psq = h["psq"]
                        qsb = sp.tile([128, SQT], BF16, tag="qsb", name="qsb")
                        nc.vector.tensor_copy(qsb[:], psq[dt][:])
                        qsw = sp.tile([128, SQT], BF16, tag="qsw", name="qsw")
                        nc.vector.stream_shuffle(qsw[:], qsb[:], SWAP32)
                        t1 = sp.tile([128, SQT], BF16, tag="t1", name="t1")
                        nc.vector.tensor_mul(t1[:], qsb[:], cos_sb[:, ss])
                        t2 = sp.tile([128, SQT], BF16, tag="t2", name="t2")
                        nc.vector.tensor_mul(t2[:], qsw[:], sin_sb[:, ss])
                        nc.vector.tensor_add(qrot[dt][:, ss], t1[:], t2[:])
                    return f

                for dt in range(2):
                    filler_q.append((0, mk_rope(dt)))

                def mk_kv(kt):
                    xt_t = xt_cache[kt]

                    def f():
                        if kt == 0:
                            h["pskv"] = pool8.tile([128, SQT], F32, tag="proj",
                                                   name="pskv", bufs=2)
                        nc.tensor.matmul(
                            h["pskv"][:],
                            wkv_ch[kt // 4][:, (kt % 4) * 128:(kt % 4 + 1) * 128],
                            xt_t[:],
                            start=(kt == 0),
                            stop=(kt == KT - 1),
                        )
                    return f

                for kt in range(KT):
                    filler_q.append((1, mk_kv(kt)))

                def rope_k():
                    pskv = h["pskv"]
                    ksb = sp.tile([64, SQT], BF16, tag="ksb", name="ksb")
                    nc.vector.tensor_copy(ksb[:], pskv[0:64, :])
                    ksw = sp.tile([64, SQT], BF16, tag="ksw", name="ksw")
                    nc.vector.stream_shuffle(ksw[:], ksb[:], SWAP32)
                    t1k = sp.tile([64, SQT], BF16, tag="t1k", name="t1k")
                    nc.vector.tensor_mul(t1k[:], ksb[:], cos_sb[0:64, ss])
                    t2k = sp.tile([64, SQT], BF16, tag="t2k", name="t2k")
                    nc.vector.tensor_mul(t2k[:], ksw[:], sin_sb[0:64, ss])
                    nc.vector.tensor_add(krot2[0][0:64, ss], t1k[:], t2k[:])
                    nc.vector.tensor_add(krot2[1][64:128, ss], t1k[:], t2k[:])

                filler_q.append((0, rope_k))

                def vtrans():
                    pskv = h["pskv"]
                    vsb = sp.tile([64, SQT], BF16, tag="vsb", name="vsb")
                    nc.vector.tensor_copy(vsb[:], pskv[64:128, :])
                    for c in range(SQT // 128):
                        j = st * 4 + c  # global sk tile index
                        pt = pool8.tile([128, 64], BF16, tag="proj", name="pt",
                                        bufs=2)
                        nc.tensor.transpose(
                            pt[:], vsb[:, c * 128:(c + 1) * 128], id_sb[:]
                        )
                        nc.vector.tensor_copy(vaug[:, j * 65: j * 65 + 64], pt[:])

                filler_q.append((1, vtrans))

            def queue_c(b, sqt):
                """Queue output-projection work for block (b,sqt) as fillers."""
                for sti in range(SQT // 128):
                    st = (b * S + sqt * SQT) // 128 + sti
                    for ot in range(DIM // SQT):
                        def mk(st=st, ot=ot, sti=sti):
                            def f():
                                pw = pool8.tile([128, SQT], F32, tag="proj",
                                                name="pw", bufs=2)
                                for jt in range(2):
                                    nc.tensor.matmul(
                                        pw[:],
                                        attnT[jt][:, st * 128:(st + 1) * 128],
                                        wo_sb[jt][:, ot * SQT:(ot + 1) * SQT],
                                        start=(jt == 0),
                                        stop=(jt == 1),
                                    )
                                osb = woutp.tile([128, SQT], BF16, tag="osb",
                                                 name="osb")
                                if (sti + ot) % 2 == 0:
                                    nc.scalar.copy(osb[:], pw[:])
                                else:
                                    nc.vector.tensor_copy(osb[:], pw[:])
                                nc.sync.dma_start(
                                    out[st * 128:(st + 1) * 128,
                                        ot * SQT:(ot + 1) * SQT],
                                    osb[:],
                                )
                            return f
                        filler_q.append((2, mk()))

            def emit_b(b, sqt):
                n_sk = 4 * (sqt + 1)
                sq0 = b * S + sqt * SQT
                for dt in range(2):  # head pair (hp=0,1 packed in PE halves)
                    po = [pool8.tile([128, SQT], F32, tag="po",
                                     name=f"po{hp}", bufs=2) for hp in range(2)]
                    pend = deque()  # pipelined PV args, depth 2

                    def emit_pv(ets, j, off):
                        jj = b * NSK + j
                        for hp in range(2):
                            nc.tensor.matmul(
                                po[hp][0:65, off:SQT],
                                vaug[:, jj * 65:(jj + 1) * 65],
                                ets[hp][:, off:SQT],
                                start=(j == 0),
                                stop=(j == n_sk - 1),
                            )

                    for j in range(n_sk):
                        sk0 = b * S + j * SKT
                        d = j - 4 * sqt
                        off = max(0, 128 * d)  # causally dead columns
                        pss = []
                        for hp in range(2):
                            ps = pool8.tile([128, SQT], F32, tag="sc",
                                            name="ps", bufs=4)
                            nc.tensor.matmul(
                                ps[:, off:SQT],
                                krot2[hp][:, sk0:sk0 + SKT],
                                qrot[dt][:, sq0 + off:sq0 + SQT],
                                start=True,
                                stop=(d < 0),
                            )
                            if d >= 0:
                                # fold causal mask into PSUM: adds
                                # -32768*max(0, sk-sq) on the 128-wide band
                                nc.tensor.matmul(
                                    ps[:, off:off + 128],
                                    ltri_sb[:],
                                    utri_sb[:],
                                    start=False,
                                    stop=True,
                                    skip_group_check=True,
                                )
                            pss.append(ps)
                        ets = []
                        for hp in range(2):
                            et = ep.tile([128, SQT], BF16, tag=f"et{hp}",
                                         name=f"et{hp}")
                            nc.scalar.activation(
                                et[:, off:SQT], pss[hp][:, off:SQT],
                                mybir.ActivationFunctionType.Exp,
                                scale=0.125,
                            )
                            ets.append(et)
                        pend.append((ets, j, off))
                        if j % 4 == 3:
                            pump(6)
                        if len(pend) > 2:
                            emit_pv(*pend.popleft())
                    while pend:
                        pump(3)
                        emit_pv(*pend.popleft())
                    for hp in range(2):
                        den = mp.tile([1, SQT], F32, tag="den", name="den")
                        nc.vector.tensor_copy(den[:], po[hp][64:65, :])
                        recip = mp.tile([1, SQT], F32, tag="recip", name="recip")
                        nc.vector.reciprocal_approx_fast(recip[:], den[:])
                        bc = mp.tile([64, SQT], F32, tag="bc", name="bc")
                        nc.gpsimd.partition_broadcast(bc[:], recip[:])
                        nc.vector.tensor_mul(
                            attnT[dt][hp * 64:(hp + 1) * 64, sq0:sq0 + SQT],
                            po[hp][0:64, :],
                            bc[:],
                        )

            # schedule: A(st) queued TWO blocks ahead of the B block whose
            # scores read its qrot, so the vector rope chain of a drained A
            # has a whole block of slack before the PE depends on it.
            queue_a(0)
            late_dmas()
            pump_all()
            queue_a(1)
            pump_all()
            queue_a(2); emit_b(0, 0); pump_all(); queue_c(0, 0)
            queue_a(3); emit_b(0, 1); pump_all(); queue_c(0, 1)
            queue_a(4); emit_b(0, 2); pump_all(); queue_c(0, 2)
            queue_a(5); emit_b(0, 3); pump_all(); queue_c(0, 3)
            queue_a(6); emit_b(1, 0); pump_all(); queue_c(1, 0)
            queue_a(7); emit_b(1, 1); pump_all(); queue_c(1, 1)
            emit_b(1, 2); queue_c(1, 2)
            emit_b(1, 3)
            queue_c(1, 3)
            pump_all()

    nc.compile()
    nc.m = get_hw_module(nc.m)
    _CACHE["nc"] = nc
    return nc


def _prep_inputs(x, freqs_cos, freqs_sin, wq, wk, wv, wo):
    bf = ml_dtypes.bfloat16
    xT = np.ascontiguousarray(x.reshape(ST, DIM).T).astype(bf)
    # expanded rope tables in [feature, seq] layout, tiled over 2 head rows;
    # sin carries the pair-swap signs (-sin on even rows, +sin on odd)
    cos64 = np.repeat(freqs_cos.T, 2, axis=0)        # [64, S]
    sin64 = np.repeat(freqs_sin.T, 2, axis=0)
    sgn = np.tile(np.array([-1.0, 1.0]), 32)[:, None]
    sin64 = sin64 * sgn
    cosE = np.tile(np.tile(cos64, (2, 1)), (1, B)).astype(bf)  # [128, ST]
    sinE = np.tile(np.tile(sin64, (2, 1)), (1, B)).astype(bf)
    ident = np.eye(64, dtype=np.float32).astype(bf)
    # causal-mask factor pair: (ltri.T @ utri)[m, n] = -32768*max(0, m-n)
    rr = np.arange(128)
    ltri = (rr[:, None] <= rr[None, :]).astype(np.float32).astype(bf)  # r<=m
    utri = ((rr[:, None] > rr[None, :]) * -32768.0).astype(np.float32).astype(bf)
    ones64 = np.ones((1, 64), dtype=np.float32)

    in_maps = []
    for c in range(N_CORES):
        wq_c = wq[c * QD:(c + 1) * QD, :]
        wk_c = wk[c * HD:(c + 1) * HD, :]
        wv_c = wv[c * HD:(c + 1) * HD, :]
        wkv_c = np.concatenate([wk_c, wv_c], axis=0)   # [128, DIM]
        wo_c = wo[:, c * QD:(c + 1) * QD]              # [DIM, 256]
        in_maps.append({
            "xt": xT,
            "wqt": np.ascontiguousarray(wq_c.T).astype(bf),
            "wkvt": np.ascontiguousarray(wkv_c.T).astype(bf),
            "wot": np.ascontiguousarray(wo_c.T).astype(bf),
            "cose": cosE,
            "sine": sinE,
            "ident": ident,
            "ltri": ltri,
            "utri": utri,
            "ones64": ones64,
        })
    return in_maps


def kernel(x, freqs_cos, freqs_sin, wq, wk, wv, wo, _trace=False, _trace_kwargs=None):
    x = np.asarray(x, dtype=np.float32)
    freqs_cos = np.asarray(freqs_cos, dtype=np.float32)
    freqs_sin = np.asarray(freqs_sin, dtype=np.float32)
    wq = np.asarray(wq, dtype=np.float32)
    wk = np.asarray(wk, dtype=np.float32)
    wv = np.asarray(wv, dtype=np.float32)
    wo = np.asarray(wo, dtype=np.float32)

    nc = _build()
    in_maps = _prep_inputs(x, freqs_cos, freqs_sin, wq, wk, wv, wo)
    kwargs = dict(_trace_kwargs or {})
    res = bass_utils.run_bass_kernel_spmd(
        nc, in_maps, core_ids=list(range(N_CORES)), trace=_trace, **kwargs
    )
    _CACHE["last_result"] = res
    acc = res.results[0]["out"].astype(np.float32)
    for c in range(1, N_CORES):
        acc += res.results[c]["out"].astype(np.float32)
    return acc.reshape(B, S, DIM)


# revision 54
# speedup vs baseline: 1.1540x; 1.0036x over previous
"""GQA attention (B=2, S=2048, D=2048, 32 q-heads / 8 kv-heads, hd=64),
tensor-parallel over the 8 kv-head groups on 8 NeuronCores.

Per-core math (core c owns kv head c and q heads 4c..4c+3):
  qT = (wq_c @ x.T), kT/vT likewise; RoPE via elementwise muls with a
  partition pair-swap done by DVE stream_shuffle and a sign-folded sin
  table; scoresT[sk,sq] = k_rot.T-layout matmul; causal masking folded
  into the scores PSUM accumulation via a pair of constant triangular
  matrices (ltri.T@utri adds -32768*max(0,sk-sq) on the 128-wide
  diagonal band) so exp output needs no post-masking; ET = exp(scoresT/8);
  out_pvT and the softmax denominator come from one matmul against
  [V | ones]; partial = attnT.T @ woT_c accumulated in fp32 and summed
  on host.

Scheduling: the PE runs its queue in order and downclocks (p-state)
whenever it idles, so the kernel aims for long contiguous PE streaks:
 - scores are software-pipelined 2 steps ahead of PV (4 PSUM banks),
 - the Scalar engine's exp throughput deficit (~0.3us per score step)
   is absorbed by pulling coarse chunks of projection/output-projection
   matmuls from a filler queue every 4th step,
 - projections run as two passes (Q then KV) over the cached x tiles so
   they need only 2 PSUM banks, shared with the output projection.
"""

from collections import deque
from contextlib import ExitStack

import ml_dtypes
import numpy as np

import concourse.bass as bass
import concourse.tile as tile
from concourse import bacc, mybir
from concourse import bass_utils
from concourse.bass_interp import get_hw_module

BF16 = mybir.dt.bfloat16
F32 = mybir.dt.float32
F32R = mybir.dt.float32r

N_CORES = 8
B, S, DIM = 2, 2048, 2048
NH, NKV, HD = 32, 8, 64          # global heads
NHC = NH // N_CORES              # q heads per core = 4
QD = NHC * HD                    # per-core q out dim = 256
ST = B * S                       # total tokens = 4096
KT = DIM // 128                  # contraction k-tiles = 16
SQT = 512                        # sq tile (matmul free dim)
SKT = 128                        # sk tile (partition dim)
NSQ = S // SQT                   # sq tiles per batch = 4
NSK = S // SKT                   # sk tiles per batch = 16

SWAP32 = [i ^ 1 for i in range(32)]  # pair-swap within 32-partition groups

_CACHE: dict = {}


def _build():
    if "nc" in _CACHE:
        return _CACHE["nc"]
    nc = bacc.Bacc(
        "TRN2",
        target_bir_lowering=False,
        debug=False,
        enable_asserts=False,
        num_devices=N_CORES,
    )
    xT = nc.dram_tensor("xt", [DIM, ST], BF16, kind="ExternalInput").ap()
    wqT = nc.dram_tensor("wqt", [DIM, QD], BF16, kind="ExternalInput").ap()
    wkvT = nc.dram_tensor("wkvt", [DIM, 2 * HD], BF16, kind="ExternalInput").ap()
    woT = nc.dram_tensor("wot", [QD, DIM], BF16, kind="ExternalInput").ap()
    cosE = nc.dram_tensor("cose", [128, ST], BF16, kind="ExternalInput").ap()
    sinE = nc.dram_tensor("sine", [128, ST], BF16, kind="ExternalInput").ap()
    ident = nc.dram_tensor("ident", [64, 64], BF16, kind="ExternalInput").ap()
    ltri = nc.dram_tensor("ltri", [128, 128], BF16, kind="ExternalInput").ap()
    utri = nc.dram_tensor("utri", [128, 128], BF16, kind="ExternalInput").ap()
    ones64 = nc.dram_tensor("ones64", [1, 64], F32, kind="ExternalInput").ap()
    out = nc.dram_tensor("out", [ST, DIM], BF16, kind="ExternalOutput").ap()

    with tile.TileContext(nc) as tc, ExitStack() as ctx:
        pers = ctx.enter_context(tc.tile_pool(name="pers", bufs=1))

        # -- persistent SBUF tensors ------------------------------------
        wq_ch = [pers.tile([128, 4 * QD], BF16, tag=f"wq{g}", name=f"wq{g}")
                 for g in range(4)]
        wkv_ch = [pers.tile([128, 4 * 2 * HD], BF16, tag=f"wkv{g}",
                            name=f"wkv{g}") for g in range(4)]
        wo_sb = [pers.tile([128, DIM], BF16, tag=f"wo{j}", name=f"wo{j}") for j in range(2)]
        cos_sb = pers.tile([128, ST], BF16, tag="cos")
        sin_sb = pers.tile([128, ST], BF16, tag="sin")
        id_sb = pers.tile([64, 64], BF16, tag="ident")
        ltri_sb = pers.tile([128, 128], BF16, tag="ltri")
        utri_sb = pers.tile([128, 128], BF16, tag="utri")
        ones64_sb = pers.tile([1, 64], F32, tag="ones64")
        qrot = [pers.tile([128, ST], BF16, tag=f"qrot{t}", name=f"qrot{t}") for t in range(2)]
        # zero-padded stationary K: krot2[hp] holds k_rot in partition half
        # hp and zeros in the other, so scores run as full K=128 matmuls
        # (K=64 quadrant matmuls pay a ~106ns fixed cost on TRN2)
        krot2 = [pers.tile([128, ST], BF16, tag=f"krot{t}", name=f"krot{t}") for t in range(2)]
        vaug = pers.tile([128, B * NSK * 65], BF16, tag="vaug")
        warm_src = pers.tile([1, 8], F32, tag="warm_src")
        warm_dst = pers.tile([2, 8], F32, tag="warm_dst")
        attnT = [pers.tile([128, ST], BF16, tag=f"attnT{t}", name=f"attnT{t}") for t in range(2)]

        # weights first: the first projection matmuls gate kernel start
        wqT_v = wqT.rearrange("(t p) d -> p t d", p=128)
        wkvT_v = wkvT.rearrange("(t p) d -> p t d", p=128)
        for g in range(4):
            gs = slice(g * 4, (g + 1) * 4)
            nc.sync.dma_start(
                wq_ch[g].rearrange("p (t d) -> p t d", t=4), wqT_v[:, gs, :]
            )
            nc.sync.dma_start(
                wkv_ch[g].rearrange("p (t d) -> p t d", t=4), wkvT_v[:, gs, :]
            )
        nc.gpsimd.memset(warm_src[:], 1.0)

        def late_dmas():
            # ones column of V_aug (col 64 of each 65-wide block); these
            # memsets go after the first x-tile DMA triggers on the gpsimd
            # queue so they don't delay the kernel start
            nc.gpsimd.memset(vaug[:, 64::65], 1.0)
            nc.gpsimd.memset(krot2[0][64:128, :], 0.0)
            nc.gpsimd.memset(krot2[1][0:64, :], 0.0)
            # issued after the first x tiles so they don't delay the start
            for j in range(2):
                nc.sync.dma_start(wo_sb[j][:], woT[j * 128:(j + 1) * 128, :])
            for g in range(4):
                gs = bass.ts(g, ST // 4)
                nc.sync.dma_start(cos_sb[:, gs], cosE[:, gs])
                nc.sync.dma_start(sin_sb[:, gs], sinE[:, gs])
            nc.sync.dma_start(id_sb[:], ident[:])
            nc.sync.dma_start(ltri_sb[:], ltri[:])
            nc.sync.dma_start(utri_sb[:], utri[:])
            nc.sync.dma_start(ones64_sb[:], ones64[:])

        # -- pools -------------------------------------------------------
        # PSUM (8 banks): proj 2 (A projections + C output proj),
        #                 sc 4 (scores pipeline depth 2), po 2 (PV accum)
        with tc.tile_pool(name="xt", bufs=32) as xp, \
             tc.tile_pool(name="stage", bufs=2) as sp, \
             tc.tile_pool(name="et", bufs=6) as ep, \
             tc.tile_pool(name="misc", bufs=2) as mp, \
             tc.tile_pool(name="wout", bufs=8) as woutp, \
             tc.tile_pool(name="ps8", bufs=1, space="PSUM") as pool8:

            xt_cache = {}
            filler_q = deque()  # (pe_weight, thunk) of deferred work

            def pump(units):
                while units > 0 and filler_q:
                    w, f = filler_q.popleft()
                    f()
                    units -= w

            def pump_all():
                while filler_q:
                    _, f = filler_q.popleft()
                    f()

            def queue_a(st):
                """Queue projection+rope work for seq tile st as fillers.

                All PSUM allocation happens lazily inside thunks so
                tag-slot round-robin order matches PE emission order."""
                ss = bass.ts(st, SQT)
                h = {}
                xt_cache.clear()
                for kt in range(KT):
                    t = xp.tile([128, SQT], BF16, name="xt_t")
                    nc.gpsimd.dma_start(
                        t[:], xT[kt * 128:(kt + 1) * 128,
                                 st * SQT:(st + 1) * SQT]
                    )
                    xt_cache[kt] = t
                # pre-warm the gpsimd partition_broadcast library: the queue
                # alternates DMA-trigger and broadcast code, and the ~5us
                # library reload otherwise lands on the po-reuse critical
                # path at the block's first normalize
                nc.gpsimd.partition_broadcast(warm_dst[:], warm_src[:])

                def mk_q(kt):
                    xt_t = xt_cache[kt]

                    def f():
                        if kt == 0:
                            h["psq"] = [pool8.tile([128, SQT], F32, tag="proj",
                                                   name="psq", bufs=2)
                                        for _ in range(2)]
                        psq = h["psq"]
                        for dt in range(2):
                            nc.tensor.matmul(
                                psq[dt][:],
                                wq_ch[kt // 4][:, (kt % 4) * QD + dt * 128:
                                               (kt % 4) * QD + (dt + 1) * 128],
                                xt_t[:],
                                start=(kt == 0),
                                stop=(kt == KT - 1),
                            )
                    return f

                for kt in range(KT):
                    filler_q.append((2, mk_q(kt)))

                def mk_rope(dt):
                    def f():
                        psq = h["psq"]
                        qsb = sp.tile([128, SQT], BF16, tag="qsb", name="qsb")
                        nc.vector.tensor_copy(qsb[:], psq[dt][:])
                        qsw = sp.tile([128, SQT], BF16, tag="qsw", name="qsw")
                        nc.vector.stream_shuffle(qsw[:], qsb[:], SWAP32)
                        t1 = sp.tile([128, SQT], BF16, tag="t1", name="t1")
                        nc.vector.tensor_mul(t1[:], qsb[:], cos_sb[:, ss])
                        t2 = sp.tile([128, SQT], BF16, tag="t2", name="t2")
                        nc.vector.tensor_mul(t2[:], qsw[:], sin_sb[:, ss])
                        nc.vector.tensor_add(qrot[dt][:, ss], t1[:], t2[:])
                    return f

                for dt in range(2):
                    filler_q.append((0, mk_rope(dt)))

                def mk_kv(kt):
                    xt_t = xt_cache[kt]

                    def f():
                        if kt == 0:
                            h["pskv"] = pool8.tile([128, SQT], F32, tag="proj",
                                                   name="pskv", bufs=2)
                        nc.tensor.matmul(
                            h["pskv"][:],
                            wkv_ch[kt // 4][:, (kt % 4) * 128:(kt % 4 + 1) * 128],
                            xt_t[:],
                            start=(kt == 0),
                            stop=(kt == KT - 1),
                        )
                    return f

                for kt in range(KT):
                    filler_q.append((1, mk_kv(kt)))

                def rope_k():
                    pskv = h["pskv"]
                    ksb = sp.tile([64, SQT], BF16, tag="ksb", name="ksb")
                    nc.vector.tensor_copy(ksb[:], pskv[0:64, :])
                    ksw = sp.tile([64, SQT], BF16, tag="ksw", name="ksw")
                    nc.vector.stream_shuffle(ksw[:], ksb[:], SWAP32)
                    t1k = sp.tile([64, SQT], BF16, tag="t1k", name="t1k")
                    nc.vector.tensor_mul(t1k[:], ksb[:], cos_sb[0:64, ss])
                    t2k = sp.tile([64, SQT], BF16, tag="t2k", name="t2k")
                    nc.vector.tensor_mul(t2k[:], ksw[:], sin_sb[0:64, ss])
                    nc.vector.tensor_add(krot2[0][0:64, ss], t1k[:], t2k[:])
                    nc.vector.tensor_add(krot2[1][64:128, ss], t1k[:], t2k[:])

                filler_q.append((0, rope_k))

                def vtrans():
                    pskv = h["pskv"]
                    vsb = sp.tile([64, SQT], BF16, tag="vsb", name="vsb")
                    nc.vector.tensor_copy(vsb[:], pskv[64:128, :])
                    for c in range(SQT // 128):
                        j = st * 4 + c  # global sk tile index
                        pt = pool8.tile([128, 64], BF16, tag="proj", name="pt",
                                        bufs=2)
                        nc.tensor.transpose(
                            pt[:], vsb[:, c * 128:(c + 1) * 128], id_sb[:]
                        )
                        nc.vector.tensor_copy(vaug[:, j * 65: j * 65 + 64], pt[:])

                filler_q.append((1, vtrans))

            def queue_c(b, sqt):
                """Queue output-projection work for block (b,sqt) as fillers."""
                for sti in range(SQT // 128):
                    st = (b * S + sqt * SQT) // 128 + sti
                    for ot in range(DIM // SQT):
                        def mk(st=st, ot=ot, sti=sti):
                            def f():
                                pw = pool8.tile([128, SQT], F32, tag="proj",
                                                name="pw", bufs=2)
                                for jt in range(2):
                                    nc.tensor.matmul(
                                        pw[:],
                                        attnT[jt][:, st * 128:(st + 1) * 128],
                                        wo_sb[jt][:, ot * SQT:(ot + 1) * SQT],
                                        start=(jt == 0),
                                        stop=(jt == 1),
                                    )
                                osb = woutp.tile([128, SQT], BF16, tag="osb",
                                                 name="osb")
                                if (sti + ot) % 2 == 0:
                                    nc.scalar.copy(osb[:], pw[:])
                                else:
                                    nc.vector.tensor_copy(osb[:], pw[:])
                                nc.sync.dma_start(
                                    out[st * 128:(st + 1) * 128,
                                        ot * SQT:(ot + 1) * SQT],
                                    osb[:],
                                )
                            return f
                        filler_q.append((2, mk()))

            def emit_b(b, sqt):
                n_sk = 4 * (sqt + 1)
                sq0 = b * S + sqt * SQT
                for dt in range(2):  # head pair (hp=0,1 packed in PE halves)
                    po = [pool8.tile([128, SQT], F32, tag="po",
                                     name=f"po{hp}", bufs=2) for hp in range(2)]
                    pend = deque()  # pipelined PV args, depth 2

                    def emit_pv(et, j, off):
                        jj = b * NSK + j
                        for hp in range(2):
                            h0 = hp * SQT
                            nc.tensor.matmul(
                                po[hp][0:65, off:SQT],
                                vaug[:, jj * 65:(jj + 1) * 65],
                                et[:, h0 + off:h0 + SQT],
                                start=(j == 0),
                                stop=(j == n_sk - 1),
                            )

                    for j in range(n_sk):
                        sk0 = b * S + j * SKT
                        d = j - 4 * sqt
                        off = max(0, 128 * d)  # causally dead columns
                        # both heads' scores go into one 2-bank PSUM tile so
                        # a single exp covers the pair
                        ps = pool8.tile([128, 2 * SQT], F32, tag="sc",
                                        name="ps", bufs=2)
                        for hp in range(2):
                            h0 = hp * SQT
                            nc.tensor.matmul(
                                ps[:, h0 + off:h0 + SQT],
                                krot2[hp][:, sk0:sk0 + SKT],
                                qrot[dt][:, sq0 + off:sq0 + SQT],
                                start=True,
                                stop=(d < 0),
                            )
                            if d >= 0:
                                # fold causal mask into PSUM: adds
                                # -32768*max(0, sk-sq) on the 128-wide band
                                nc.tensor.matmul(
                                    ps[:, h0 + off:h0 + off + 128],
                                    ltri_sb[:],
                                    utri_sb[:],
                                    start=False,
                                    stop=True,
                                    skip_group_check=True,
                                )
                        et = ep.tile([128, 2 * SQT], BF16, tag="et", name="et")
                        if off == 0:
                            nc.scalar.activation(
                                et[:], ps[:],
                                mybir.ActivationFunctionType.Exp,
                                scale=0.125,
                            )
                        else:
                            for hp in range(2):
                                h0 = hp * SQT
                                nc.scalar.activation(
                                    et[:, h0 + off:h0 + SQT],
                                    ps[:, h0 + off:h0 + SQT],
                                    mybir.ActivationFunctionType.Exp,
                                    scale=0.125,
                                )
                        pend.append((et, j, off))
                        if j % 4 == 3:
                            pump(6)
                        if len(pend) > 2:
                            emit_pv(*pend.popleft())
                    while pend:
                        pump(3)
                        emit_pv(*pend.popleft())
                    for hp in range(2):
                        den = mp.tile([1, SQT], F32, tag="den", name="den")
                        nc.vector.tensor_copy(den[:], po[hp][64:65, :])
                        recip = mp.tile([1, SQT], F32, tag="recip", name="recip")
                        nc.vector.reciprocal_approx_fast(recip[:], den[:])
                        bc = mp.tile([64, SQT], F32, tag="bc", name="bc")
                        nc.gpsimd.partition_broadcast(bc[:], recip[:])
                        nc.vector.tensor_mul(
                            attnT[dt][hp * 64:(hp + 1) * 64, sq0:sq0 + SQT],
                            po[hp][0:64, :],
                            bc[:],
                        )

            # schedule: A(st) queued TWO blocks ahead of the B block whose
            # scores read its qrot, so the vector rope chain of a drained A
            # has a whole block of slack before the PE depends on it.
            queue_a(0)
            late_dmas()
            pump_all()
            queue_a(1)
            pump_all()
            queue_a(2); emit_b(0, 0); pump_all(); queue_c(0, 0)
            queue_a(3); emit_b(0, 1); pump_all(); queue_c(0, 1)
            queue_a(4); emit_b(0, 2); pump_all(); queue_c(0, 2)
            queue_a(5); emit_b(0, 3); pump_all(); queue_c(0, 3)
            queue_a(6); emit_b(1, 0); pump_all(); queue_c(1, 0)
            queue_a(7); emit_b(1, 1); pump_all(); queue_c(1, 1)
            emit_b(1, 2); queue_c(1, 2)
            emit_b(1, 3)
            queue_c(1, 3)
            pump_all()

    nc.compile()
    nc.m = get_hw_module(nc.m)
    _CACHE["nc"] = nc
    return nc


def _prep_inputs(x, freqs_cos, freqs_sin, wq, wk, wv, wo):
    bf = ml_dtypes.bfloat16
    xT = np.ascontiguousarray(x.reshape(ST, DIM).T).astype(bf)
    # expanded rope tables in [feature, seq] layout, tiled over 2 head rows;
    # sin carries the pair-swap signs (-sin on even rows, +sin on odd)
    cos64 = np.repeat(freqs_cos.T, 2, axis=0)        # [64, S]
    sin64 = np.repeat(freqs_sin.T, 2, axis=0)
    sgn = np.tile(np.array([-1.0, 1.0]), 32)[:, None]
    sin64 = sin64 * sgn
    cosE = np.tile(np.tile(cos64, (2, 1)), (1, B)).astype(bf)  # [128, ST]
    sinE = np.tile(np.tile(sin64, (2, 1)), (1, B)).astype(bf)
    ident = np.eye(64, dtype=np.float32).astype(bf)
    # causal-mask factor pair: (ltri.T @ utri)[m, n] = -32768*max(0, m-n)
    rr = np.arange(128)
    ltri = (rr[:, None] <= rr[None, :]).astype(np.float32).astype(bf)  # r<=m
    utri = ((rr[:, None] > rr[None, :]) * -32768.0).astype(np.float32).astype(bf)
    ones64 = np.ones((1, 64), dtype=np.float32)

    in_maps = []
    for c in range(N_CORES):
        wq_c = wq[c * QD:(c + 1) * QD, :]
        wk_c = wk[c * HD:(c + 1) * HD, :]
        wv_c = wv[c * HD:(c + 1) * HD, :]
        wkv_c = np.concatenate([wk_c, wv_c], axis=0)   # [128, DIM]
        wo_c = wo[:, c * QD:(c + 1) * QD]              # [DIM, 256]
        in_maps.append({
            "xt": xT,
            "wqt": np.ascontiguousarray(wq_c.T).astype(bf),
            "wkvt": np.ascontiguousarray(wkv_c.T).astype(bf),
            "wot": np.ascontiguousarray(wo_c.T).astype(bf),
            "cose": cosE,
            "sine": sinE,
            "ident": ident,
            "ltri": ltri,
            "utri": utri,
            "ones64": ones64,
        })
    return in_maps


def kernel(x, freqs_cos, freqs_sin, wq, wk, wv, wo, _trace=False, _trace_kwargs=None):
    x = np.asarray(x, dtype=np.float32)
    freqs_cos = np.asarray(freqs_cos, dtype=np.float32)
    freqs_sin = np.asarray(freqs_sin, dtype=np.float32)
    wq = np.asarray(wq, dtype=np.float32)
    wk = np.asarray(wk, dtype=np.float32)
    wv = np.asarray(wv, dtype=np.float32)
    wo = np.asarray(wo, dtype=np.float32)

    nc = _build()
    in_maps = _prep_inputs(x, freqs_cos, freqs_sin, wq, wk, wv, wo)
    kwargs = dict(_trace_kwargs or {})
    res = bass_utils.run_bass_kernel_spmd(
        nc, in_maps, core_ids=list(range(N_CORES)), trace=_trace, **kwargs
    )
    _CACHE["last_result"] = res
    acc = res.results[0]["out"].astype(np.float32)
    for c in range(1, N_CORES):
        acc += res.results[c]["out"].astype(np.float32)
    return acc.reshape(B, S, DIM)


# revision 57
# speedup vs baseline: 1.2012x; 1.0409x over previous
"""GQA attention (B=2, S=2048, D=2048, 32 q-heads / 8 kv-heads, hd=64),
tensor-parallel over the 8 kv-head groups on 8 NeuronCores.

Per-core math (core c owns kv head c and q heads 4c..4c+3):
  qT = (wq_c @ x.T), kT/vT likewise; RoPE via elementwise muls with a
  partition pair-swap done by DVE stream_shuffle and a sign-folded sin
  table; scoresT[sk,sq] = k_rot.T-layout matmul; causal masking folded
  into the scores PSUM accumulation via a pair of constant triangular
  matrices (ltri.T@utri adds -32768*max(0,sk-sq) on the 128-wide
  diagonal band) so exp output needs no post-masking; ET = exp(scoresT/8);
  out_pvT and the softmax denominator come from one matmul against
  [V | ones]; partial = attnT.T @ woT_c accumulated in fp32 and summed
  on host.

Scheduling: the PE runs its queue in order and downclocks (p-state)
whenever it idles, so the kernel aims for long contiguous PE streaks:
 - scores are software-pipelined 2 steps ahead of PV (4 PSUM banks),
 - the Scalar engine's exp throughput deficit (~0.3us per score step)
   is absorbed by pulling coarse chunks of projection/output-projection
   matmuls from a filler queue every 4th step,
 - projections run as two passes (Q then KV) over the cached x tiles so
   they need only 2 PSUM banks, shared with the output projection.
"""

from collections import deque
from contextlib import ExitStack

import ml_dtypes
import numpy as np

import concourse.bass as bass
import concourse.tile as tile
from concourse import bacc, mybir
from concourse import bass_utils
from concourse.bass_interp import get_hw_module

BF16 = mybir.dt.bfloat16
F32 = mybir.dt.float32
F32R = mybir.dt.float32r

N_CORES = 8
B, S, DIM = 2, 2048, 2048
NH, NKV, HD = 32, 8, 64          # global heads
NHC = NH // N_CORES              # q heads per core = 4
QD = NHC * HD                    # per-core q out dim = 256
ST = B * S                       # total tokens = 4096
KT = DIM // 128                  # contraction k-tiles = 16
SQT = 512                        # sq tile (matmul free dim)
SKT = 128                        # sk tile (partition dim)
NSQ = S // SQT                   # sq tiles per batch = 4
NSK = S // SKT                   # sk tiles per batch = 16

SWAP32 = [i ^ 1 for i in range(32)]  # pair-swap within 32-partition groups

_CACHE: dict = {}


def _build():
    if "nc" in _CACHE:
        return _CACHE["nc"]
    nc = bacc.Bacc(
        "TRN2",
        target_bir_lowering=False,
        debug=False,
        enable_asserts=False,
        num_devices=N_CORES,
    )
    xT = nc.dram_tensor("xt", [DIM, ST], BF16, kind="ExternalInput").ap()
    wqT = nc.dram_tensor("wqt", [DIM, QD], BF16, kind="ExternalInput").ap()
    wkvT = nc.dram_tensor("wkvt", [DIM, 2 * HD], BF16, kind="ExternalInput").ap()
    woT = nc.dram_tensor("wot", [QD, DIM], BF16, kind="ExternalInput").ap()
    cosE = nc.dram_tensor("cose", [128, ST], BF16, kind="ExternalInput").ap()
    sinE = nc.dram_tensor("sine", [128, ST], BF16, kind="ExternalInput").ap()
    ident = nc.dram_tensor("ident", [64, 64], BF16, kind="ExternalInput").ap()
    ltri = nc.dram_tensor("ltri", [128, 128], BF16, kind="ExternalInput").ap()
    utri = nc.dram_tensor("utri", [128, 128], BF16, kind="ExternalInput").ap()
    ones64 = nc.dram_tensor("ones64", [1, 64], F32, kind="ExternalInput").ap()
    out = nc.dram_tensor("out", [ST, DIM], BF16, kind="ExternalOutput").ap()

    with tile.TileContext(nc) as tc, ExitStack() as ctx:
        pers = ctx.enter_context(tc.tile_pool(name="pers", bufs=1))

        # -- persistent SBUF tensors ------------------------------------
        wq_ch = [pers.tile([128, 4 * QD], BF16, tag=f"wq{g}", name=f"wq{g}")
                 for g in range(4)]
        wkv_ch = [pers.tile([128, 4 * 2 * HD], BF16, tag=f"wkv{g}",
                            name=f"wkv{g}") for g in range(4)]
        wo_sb = [pers.tile([128, DIM], BF16, tag=f"wo{j}", name=f"wo{j}") for j in range(2)]
        cos_sb = pers.tile([128, ST], BF16, tag="cos")
        sin_sb = pers.tile([128, ST], BF16, tag="sin")
        id_sb = pers.tile([64, 64], BF16, tag="ident")
        ltri_sb = pers.tile([128, 128], BF16, tag="ltri")
        utri_sb = pers.tile([128, 128], BF16, tag="utri")
        ones64_sb = pers.tile([1, 64], F32, tag="ones64")
        qrot = [pers.tile([128, ST], BF16, tag=f"qrot{t}", name=f"qrot{t}") for t in range(2)]
        # zero-padded stationary K: krot2[hp] holds k_rot in partition half
        # hp and zeros in the other, so scores run as full K=128 matmuls
        # (K=64 quadrant matmuls pay a ~106ns fixed cost on TRN2)
        krot2 = [pers.tile([128, ST], BF16, tag=f"krot{t}", name=f"krot{t}") for t in range(2)]
        vaug = pers.tile([128, B * NSK * 65], BF16, tag="vaug")
        warm_src = pers.tile([1, 8], F32, tag="warm_src")
        warm_dst = pers.tile([2, 8], F32, tag="warm_dst")
        attnT = [pers.tile([128, ST], BF16, tag=f"attnT{t}", name=f"attnT{t}") for t in range(2)]

        # weights first: the first projection matmuls gate kernel start
        wqT_v = wqT.rearrange("(t p) d -> p t d", p=128)
        wkvT_v = wkvT.rearrange("(t p) d -> p t d", p=128)
        xT_v = xT.rearrange("(t p) d -> p t d", p=128)
        for g in range(4):
            gs = slice(g * 4, (g + 1) * 4)
            nc.sync.dma_start(
                wq_ch[g].rearrange("p (t d) -> p t d", t=4), wqT_v[:, gs, :]
            )
            nc.sync.dma_start(
                wkv_ch[g].rearrange("p (t d) -> p t d", t=4), wkvT_v[:, gs, :]
            )
        nc.gpsimd.memset(warm_src[:], 1.0)

        def late_dmas():
            # ones column of V_aug (col 64 of each 65-wide block); these
            # memsets go after the first x-tile DMA triggers on the gpsimd
            # queue so they don't delay the kernel start
            nc.gpsimd.memset(vaug[:, 64::65], 1.0)
            nc.gpsimd.memset(krot2[0][64:128, :], 0.0)
            nc.gpsimd.memset(krot2[1][0:64, :], 0.0)
            # issued after the first x tiles so they don't delay the start
            for j in range(2):
                nc.sync.dma_start(wo_sb[j][:], woT[j * 128:(j + 1) * 128, :])
            for g in range(4):
                gs = bass.ts(g, ST // 4)
                nc.sync.dma_start(cos_sb[:, gs], cosE[:, gs])
                nc.sync.dma_start(sin_sb[:, gs], sinE[:, gs])
            nc.sync.dma_start(id_sb[:], ident[:])
            nc.sync.dma_start(ltri_sb[:], ltri[:])
            nc.sync.dma_start(utri_sb[:], utri[:])
            nc.sync.dma_start(ones64_sb[:], ones64[:])

        # -- pools -------------------------------------------------------
        # PSUM (8 banks): proj 2 (A projections + C output proj),
        #                 sc 4 (scores pipeline depth 2), po 2 (PV accum)
        with tc.tile_pool(name="xt", bufs=8) as xp, \
             tc.tile_pool(name="stage", bufs=2) as sp, \
             tc.tile_pool(name="et", bufs=6) as ep, \
             tc.tile_pool(name="misc", bufs=2) as mp, \
             tc.tile_pool(name="wout", bufs=8) as woutp, \
             tc.tile_pool(name="ps8", bufs=1, space="PSUM") as pool8:

            xt_cache = {}
            filler_q = deque()  # (pe_weight, thunk) of deferred work

            def pump(units):
                while units > 0 and filler_q:
                    w, f = filler_q.popleft()
                    f()
                    units -= w

            def pump_all():
                while filler_q:
                    _, f = filler_q.popleft()
                    f()

            def queue_a(st):
                """Queue projection+rope work for seq tile st as fillers.

                All PSUM allocation happens lazily inside thunks so
                tag-slot round-robin order matches PE emission order."""
                ss = bass.ts(st, SQT)
                h = {}
                xt_cache.clear()
                for g4 in range(4):
                    t = xp.tile([128, 4 * SQT], BF16, name="xt_t")
                    nc.gpsimd.dma_start(
                        t.rearrange("p (t d) -> p t d", t=4),
                        xT_v[:, g4 * 4:(g4 + 1) * 4,
                             st * SQT:(st + 1) * SQT],
                    )
                    for c in range(4):
                        xt_cache[g4 * 4 + c] = t[:, c * SQT:(c + 1) * SQT]
                # pre-warm the gpsimd partition_broadcast library: the queue
                # alternates DMA-trigger and broadcast code, and the ~5us
                # library reload otherwise lands on the po-reuse critical
                # path at the block's first normalize
                nc.gpsimd.partition_broadcast(warm_dst[:], warm_src[:])

                def mk_q(kt):
                    xt_t = xt_cache[kt]

                    def f():
                        if kt == 0:
                            h["psq"] = [pool8.tile([128, SQT], F32, tag="proj",
                                                   name="psq", bufs=2)
                                        for _ in range(2)]
                        psq = h["psq"]
                        for dt in range(2):
                            nc.tensor.matmul(
                                psq[dt][:],
                                wq_ch[kt // 4][:, (kt % 4) * QD + dt * 128:
                                               (kt % 4) * QD + (dt + 1) * 128],
                                xt_t[:],
                                start=(kt == 0),
                                stop=(kt == KT - 1),
                            )
                    return f

                for kt in range(KT):
                    filler_q.append((2, mk_q(kt)))

                def mk_rope(dt):
                    def f():
                        psq = h["psq"]
                        qsb = sp.tile([128, SQT], BF16, tag="qsb", name="qsb")
                        nc.vector.tensor_copy(qsb[:], psq[dt][:])
                        qsw = sp.tile([128, SQT], BF16, tag="qsw", name="qsw")
                        nc.vector.stream_shuffle(qsw[:], qsb[:], SWAP32)
                        t1 = sp.tile([128, SQT], BF16, tag="t1", name="t1")
                        nc.vector.tensor_mul(t1[:], qsb[:], cos_sb[:, ss])
                        t2 = sp.tile([128, SQT], BF16, tag="t2", name="t2")
                        nc.vector.tensor_mul(t2[:], qsw[:], sin_sb[:, ss])
                        nc.vector.tensor_add(qrot[dt][:, ss], t1[:], t2[:])
                    return f

                for dt in range(2):
                    filler_q.append((0, mk_rope(dt)))

                def mk_kv(kt):
                    xt_t = xt_cache[kt]

                    def f():
                        if kt == 0:
                            h["pskv"] = pool8.tile([128, SQT], F32, tag="proj",
                                                   name="pskv", bufs=2)
                        nc.tensor.matmul(
                            h["pskv"][:],
                            wkv_ch[kt // 4][:, (kt % 4) * 128:(kt % 4 + 1) * 128],
                            xt_t[:],
                            start=(kt == 0),
                            stop=(kt == KT - 1),
                        )
                    return f

                for kt in range(KT):
                    filler_q.append((1, mk_kv(kt)))

                def rope_k():
                    pskv = h["pskv"]
                    ksb = sp.tile([64, SQT], BF16, tag="ksb", name="ksb")
                    nc.vector.tensor_copy(ksb[:], pskv[0:64, :])
                    ksw = sp.tile([64, SQT], BF16, tag="ksw", name="ksw")
                    nc.vector.stream_shuffle(ksw[:], ksb[:], SWAP32)
                    t1k = sp.tile([64, SQT], BF16, tag="t1k", name="t1k")
                    nc.vector.tensor_mul(t1k[:], ksb[:], cos_sb[0:64, ss])
                    t2k = sp.tile([64, SQT], BF16, tag="t2k", name="t2k")
                    nc.vector.tensor_mul(t2k[:], ksw[:], sin_sb[0:64, ss])
                    nc.vector.tensor_add(krot2[0][0:64, ss], t1k[:], t2k[:])
                    nc.vector.tensor_add(krot2[1][64:128, ss], t1k[:], t2k[:])

                filler_q.append((0, rope_k))

                def vtrans():
                    pskv = h["pskv"]
                    vsb = sp.tile([64, SQT], BF16, tag="vsb", name="vsb")
                    nc.vector.tensor_copy(vsb[:], pskv[64:128, :])
                    for c in range(SQT // 128):
                        j = st * 4 + c  # global sk tile index
                        pt = pool8.tile([128, 64], BF16, tag="proj", name="pt",
                                        bufs=2)
                        nc.tensor.transpose(
                            pt[:], vsb[:, c * 128:(c + 1) * 128], id_sb[:]
                        )
                        nc.vector.tensor_copy(vaug[:, j * 65: j * 65 + 64], pt[:])

                filler_q.append((1, vtrans))

            def queue_c(b, sqt):
                """Queue output-projection work for block (b,sqt) as fillers."""
                for sti in range(SQT // 128):
                    st = (b * S + sqt * SQT) // 128 + sti
                    for ot in range(DIM // SQT):
                        def mk(st=st, ot=ot, sti=sti):
                            def f():
                                pw = pool8.tile([128, SQT], F32, tag="proj",
                                                name="pw", bufs=2)
                                for jt in range(2):
                                    nc.tensor.matmul(
                                        pw[:],
                                        attnT[jt][:, st * 128:(st + 1) * 128],
                                        wo_sb[jt][:, ot * SQT:(ot + 1) * SQT],
                                        start=(jt == 0),
                                        stop=(jt == 1),
                                    )
                                osb = woutp.tile([128, SQT], BF16, tag="osb",
                                                 name="osb")
                                if (sti + ot) % 2 == 0:
                                    nc.scalar.copy(osb[:], pw[:])
                                else:
                                    nc.vector.tensor_copy(osb[:], pw[:])
                                nc.sync.dma_start(
                                    out[st * 128:(st + 1) * 128,
                                        ot * SQT:(ot + 1) * SQT],
                                    osb[:],
                                )
                            return f
                        filler_q.append((2, mk()))

            def emit_b(b, sqt):
                n_sk = 4 * (sqt + 1)
                sq0 = b * S + sqt * SQT
                for dt in range(2):  # head pair (hp=0,1 packed in PE halves)
                    po = [pool8.tile([128, SQT], F32, tag="po",
                                     name=f"po{hp}", bufs=2) for hp in range(2)]
                    pend = deque()  # pipelined PV args, depth 2

                    def emit_pv(et, j, off):
                        jj = b * NSK + j
                        for hp in range(2):
                            h0 = hp * SQT
                            nc.tensor.matmul(
                                po[hp][0:65, off:SQT],
                                vaug[:, jj * 65:(jj + 1) * 65],
                                et[:, h0 + off:h0 + SQT],
                                start=(j == 0),
                                stop=(j == n_sk - 1),
                            )

                    for j in range(n_sk):
                        sk0 = b * S + j * SKT
                        d = j - 4 * sqt
                        off = max(0, 128 * d)  # causally dead columns
                        # both heads' scores go into one 2-bank PSUM tile so
                        # a single exp covers the pair
                        ps = pool8.tile([128, 2 * SQT], F32, tag="sc",
                                        name="ps", bufs=2)
                        for hp in range(2):
                            h0 = hp * SQT
                            nc.tensor.matmul(
                                ps[:, h0 + off:h0 + SQT],
                                krot2[hp][:, sk0:sk0 + SKT],
                                qrot[dt][:, sq0 + off:sq0 + SQT],
                                start=True,
                                stop=(d < 0),
                            )
                            if d >= 0:
                                # fold causal mask into PSUM: adds
                                # -32768*max(0, sk-sq) on the 128-wide band
                                nc.tensor.matmul(
                                    ps[:, h0 + off:h0 + off + 128],
                                    ltri_sb[:],
                                    utri_sb[:],
                                    start=False,
                                    stop=True,
                                    skip_group_check=True,
                                )
                        et = ep.tile([128, 2 * SQT], BF16, tag="et", name="et")
                        if off == 0:
                            nc.scalar.activation(
                                et[:], ps[:],
                                mybir.ActivationFunctionType.Exp,
                                scale=0.125,
                            )
                        else:
                            for hp in range(2):
                                h0 = hp * SQT
                                nc.scalar.activation(
                                    et[:, h0 + off:h0 + SQT],
                                    ps[:, h0 + off:h0 + SQT],
                                    mybir.ActivationFunctionType.Exp,
                                    scale=0.125,
                                )
                        pend.append((et, j, off))
                        if j % 4 == 3:
                            pump(6)
                        if len(pend) > 2:
                            emit_pv(*pend.popleft())
                    while pend:
                        pump(3)
                        emit_pv(*pend.popleft())
                    for hp in range(2):
                        den = mp.tile([1, SQT], F32, tag="den", name="den")
                        nc.vector.tensor_copy(den[:], po[hp][64:65, :])
                        recip = mp.tile([1, SQT], F32, tag="recip", name="recip")
                        nc.vector.reciprocal_approx_fast(recip[:], den[:])
                        bc = mp.tile([64, SQT], F32, tag="bc", name="bc")
                        nc.gpsimd.partition_broadcast(bc[:], recip[:])
                        nc.vector.tensor_mul(
                            attnT[dt][hp * 64:(hp + 1) * 64, sq0:sq0 + SQT],
                            po[hp][0:64, :],
                            bc[:],
                        )

            # schedule: A(st) queued TWO blocks ahead of the B block whose
            # scores read its qrot, so the vector rope chain of a drained A
            # has a whole block of slack before the PE depends on it.
            queue_a(0)
            late_dmas()
            pump_all()
            queue_a(1)
            pump_all()
            queue_a(2); emit_b(0, 0); pump_all(); queue_c(0, 0)
            queue_a(3); emit_b(0, 1); pump_all(); queue_c(0, 1)
            queue_a(4); emit_b(0, 2); pump_all(); queue_c(0, 2)
            queue_a(5); emit_b(0, 3); pump_all(); queue_c(0, 3)
            queue_a(6); emit_b(1, 0); pump_all(); queue_c(1, 0)
            queue_a(7); emit_b(1, 1); pump_all(); queue_c(1, 1)
            emit_b(1, 2); queue_c(1, 2)
            emit_b(1, 3)
            queue_c(1, 3)
            pump_all()

    nc.compile()
    nc.m = get_hw_module(nc.m)
    _CACHE["nc"] = nc
    return nc


def _prep_inputs(x, freqs_cos, freqs_sin, wq, wk, wv, wo):
    bf = ml_dtypes.bfloat16
    xT = np.ascontiguousarray(x.reshape(ST, DIM).T).astype(bf)
    # expanded rope tables in [feature, seq] layout, tiled over 2 head rows;
    # sin carries the pair-swap signs (-sin on even rows, +sin on odd)
    cos64 = np.repeat(freqs_cos.T, 2, axis=0)        # [64, S]
    sin64 = np.repeat(freqs_sin.T, 2, axis=0)
    sgn = np.tile(np.array([-1.0, 1.0]), 32)[:, None]
    sin64 = sin64 * sgn
    cosE = np.tile(np.tile(cos64, (2, 1)), (1, B)).astype(bf)  # [128, ST]
    sinE = np.tile(np.tile(sin64, (2, 1)), (1, B)).astype(bf)
    ident = np.eye(64, dtype=np.float32).astype(bf)
    # causal-mask factor pair: (ltri.T @ utri)[m, n] = -32768*max(0, m-n)
    rr = np.arange(128)
    ltri = (rr[:, None] <= rr[None, :]).astype(np.float32).astype(bf)  # r<=m
    utri = ((rr[:, None] > rr[None, :]) * -32768.0).astype(np.float32).astype(bf)
    ones64 = np.ones((1, 64), dtype=np.float32)

    in_maps = []
    for c in range(N_CORES):
        wq_c = wq[c * QD:(c + 1) * QD, :]
        wk_c = wk[c * HD:(c + 1) * HD, :]
        wv_c = wv[c * HD:(c + 1) * HD, :]
        wkv_c = np.concatenate([wk_c, wv_c], axis=0)   # [128, DIM]
        wo_c = wo[:, c * QD:(c + 1) * QD]              # [DIM, 256]
        in_maps.append({
            "xt": xT,
            "wqt": np.ascontiguousarray(wq_c.T).astype(bf),
            "wkvt": np.ascontiguousarray(wkv_c.T).astype(bf),
            "wot": np.ascontiguousarray(wo_c.T).astype(bf),
            "cose": cosE,
            "sine": sinE,
            "ident": ident,
            "ltri": ltri,
            "utri": utri,
            "ones64": ones64,
        })
    return in_maps


def kernel(x, freqs_cos, freqs_sin, wq, wk, wv, wo, _trace=False, _trace_kwargs=None):
    x = np.asarray(x, dtype=np.float32)
    freqs_cos = np.asarray(freqs_cos, dtype=np.float32)
    freqs_sin = np.asarray(freqs_sin, dtype=np.float32)
    wq = np.asarray(wq, dtype=np.float32)
    wk = np.asarray(wk, dtype=np.float32)
    wv = np.asarray(wv, dtype=np.float32)
    wo = np.asarray(wo, dtype=np.float32)

    nc = _build()
    in_maps = _prep_inputs(x, freqs_cos, freqs_sin, wq, wk, wv, wo)
    kwargs = dict(_trace_kwargs or {})
    res = bass_utils.run_bass_kernel_spmd(
        nc, in_maps, core_ids=list(range(N_CORES)), trace=_trace, **kwargs
    )
    _CACHE["last_result"] = res
    acc = res.results[0]["out"].astype(np.float32)
    for c in range(1, N_CORES):
        acc += res.results[c]["out"].astype(np.float32)
    return acc.reshape(B, S, DIM)
